# revision 44
# baseline (speedup 1.0000x reference)
"""Deductron kernel for Trainium2, 8 NeuronCores, time-sharded.

Math (matching the reference):
    h = sigmoid(W1 @ x + B1); left, right = h[:128], h[128:]
    a_t = left_t * right_t; b_t = 1 - left_t
    u_0 = 0; u_t = a_{t-1} * u_{t-1} + b_{t-1}   (z[:, t] = u_t)
    out = 1 - sigmoid(W2 @ z + B2) = sigmoid(-(W2 @ z + B2))

Sharding: the 65536-frame time axis is split into 8 chunks of 8192 plus a
128-frame left washout halo per core (measured worst-case prod(a) over any
128-step boundary window is ~e^-182, so the recurrence state forgets its
initial condition well inside the halo; core 0's halo input is zero-padded
and its halo b is scaled by 0 so the state stays exactly 0).  HALO=32 was
tried and reverted: it intermittently triggered a ~10us slower execution
mode (likely SBUF alignment of the 1024-col block starts).

Key implementation points (v2 -- ACT-batched; 61.7us -> ~55us):
  * GEMM1 runs in fp8e4 (e4m3) with MatmulPerfMode.DoubleRow; W1 pre-scaled
    by 8 on host (avoids fp8 subnormals); the h-activation applies scale=1/8.
  * The Activation engine is the bottleneck (1 elem/cycle/lane @1.2GHz plus
    ~150ns fixed per instruction).  PSUM is laid out h-major -- four 2-bank
    pools psG0/psG1/psO0/psO1, one per distinct sigmoid bias -- so every
    sigmoid instruction spans 1024 columns (4 instrs/pair instead of 8),
    cutting ACT to ~4.0us/pair (measured back-to-back through the steady
    state).  Per-h pools keep GEMM(p+1) pipelined behind ACT-h(p) despite
    bufs=1 (tile deps are whole-tile).
  * Prologue: constants travel in TWO DMAs (fp8: W1|halo-x, f16: W2|biases,
    widened to f32 once on DVE); pair-0's input is split into two
    half-loads, and pairs 0-1 each use four borrowed single-t PSUM tiles
    (psO rings are idle until phase_c(0) at p=2) so 512-col sigmoids fire
    per 2-matmul group during the mid-DVFS ramp; a dummy 1-col sigmoid
    preloads the 1283ns activation table during the preamble; 7 throwaway
    matmuls climb the PE's DVFS ladder (full clock needs ~9us of
    sustained activity).
  * b = 1-left on GpSimd (a on GpSimd measured 2-4us -- Q7 tensor_tensor
    is far below roofline); a = left*right and the recurrence scan
    (tensor_tensor_scan, fp32 state, ~2.3ns/col on HW) on DVE.
  * Frames processed in PAIRS of 512-col tiles: one contiguous 512KB input
    DMA and one 512KB store per pair (DMA issue costs ~600ns serial on the
    issuing queue; completion semaphores add 900ns).
  * Drain: pair-6's out-GEMMs run on the psG rings (free after the last
    h-acts) and pair-7's on the psO rings (free after out-5), avoiding
    psO WAR stalls and scheduler head-of-line blocking; pair-7's a/scan
    are split in halves and its out-acts/stores go per 512-col quarter.
"""

import sys

for _p in ("/opt/trn_rl_repo", "/opt/pypackages"):
    if _p not in sys.path:
        sys.path.append(_p)

import numpy as np
import ml_dtypes

# Problem constants (hardcoded per contract).
INPUT_LEN = 512
N_MEM = 128
OUT_LEN = 256
T_TOTAL = 65536
N_CORES = 8
T_LOC = T_TOTAL // N_CORES   # 8192 owned frames per core
HALO = 128                   # washout halo (see module docstring)
TW = 512                     # column tile width (one PSUM bank of fp32)
NPAIR = T_LOC // (2 * TW)    # 8 pairs of owned tiles per core
W_IN = HALO + T_LOC          # 8256
W1_SCALE = 8.0               # host multiplies W1 by this; ACT applies 1/8

F16_NP = np.float16
F8_NP = ml_dtypes.float8_e4m3fn


def _build_nc():
    import concourse.tile as tile
    from concourse import bacc, mybir
    from contextlib import ExitStack

    F32 = mybir.dt.float32
    F16 = mybir.dt.float16
    F8 = mybir.dt.float8e4
    SIG = mybir.ActivationFunctionType.Sigmoid
    MUL = mybir.AluOpType.mult
    ADD = mybir.AluOpType.add
    DR = mybir.MatmulPerfMode.DoubleRow

    nc = bacc.Bacc()
    # DRAM layouts are host-packed so every DMA is fully contiguous.
    # c8[:, 0:1024] = w1 packed; c8[:, 1024:1280] = halo x packed.
    c8 = nc.dram_tensor("c8", [128, 1024 + 4 * HALO], F8, kind="ExternalInput")
    # c16[:, 0:256] = w2 packed; c16[:, 256:264] = biases
    # [B1a, B1b, -B2a, -B2b, bscale, 0, 0, 0].
    c16 = nc.dram_tensor("c16", [128, 264], F16, kind="ExternalInput")
    x_main = nc.dram_tensor("x_main", [NPAIR, 128, 4096], F8, kind="ExternalInput")
    out = nc.dram_tensor("out", [NPAIR, 128, 2048], F16, kind="ExternalOutput")

    with ExitStack() as ctx:
        tc = ctx.enter_context(tile.TileContext(nc))
        singles = ctx.enter_context(tc.tile_pool(name="singles", bufs=1))
        xpool = ctx.enter_context(tc.tile_pool(name="xpool", bufs=4))
        hpool = ctx.enter_context(tc.tile_pool(name="hpool", bufs=4))
        opool = ctx.enter_context(tc.tile_pool(name="opool", bufs=6))
        # One 2-bank PSUM pool per distinct sigmoid bias: h-GEMM halves
        # (B1a/B1b) and out-GEMM halves (-B2a/-B2b).  Each tile holds both
        # 512-col t-tiles of a pair adjacently -> 1024-col ACT instructions.
        psG0 = ctx.enter_context(tc.tile_pool(name="psG0", bufs=1, space="PSUM"))
        psG1 = ctx.enter_context(tc.tile_pool(name="psG1", bufs=1, space="PSUM"))
        psO0 = ctx.enter_context(tc.tile_pool(name="psO0", bufs=1, space="PSUM"))
        psO1 = ctx.enter_context(tc.tile_pool(name="psO1", bufs=1, space="PSUM"))

        # Persistent recurrence buffers. a_buf/b_buf are written at a +1
        # column offset (a_buf[:, p] = a at input column p-1) so the scan
        # output z[:, p] = u at column p directly.
        a_buf = singles.tile([N_MEM, W_IN + 1], F16)
        b_buf = singles.tile([N_MEM, W_IN + 1], F16)
        z_buf = singles.tile([N_MEM, W_IN], F16)

        # ---- constants: two packed DMAs; pair-0 input is issue #2 so its
        # data lands as early as possible (each dma_start serializes ~600ns
        # on the Sync queue).
        c8_sb = singles.tile([128, 1024 + 4 * HALO], F8)
        nc.sync.dma_start(out=c8_sb, in_=c8[:])
        # w1_sb[p, c, i, h, m] = 8*W1[h*128+m, c*256 + i*128 + p]
        w1_sb = c8_sb[:, 0:1024].rearrange("p (c i h m) -> p c i h m", c=2, i=2, h=2)
        xh_sb = c8_sb[:, 1024:1024 + 4 * HALO].rearrange(
            "p (c i w) -> p c i w", c=2, i=2)

        xmr = x_main[:].rearrange("q p (t c i w) -> q p t c i w", t=2, c=2, i=2)
        outr = out[:].rearrange("q p (h w) -> q p h w", h=2)

        # Biases ride the small c16 DMA -- issue #2 so the halo activations
        # aren't bias-gated.  Pair 0 arrives as two half-loads so its first
        # 512-col GEMM (and sigmoid) starts before the full pair lands.
        c16_sb = singles.tile([128, 264], F16)
        nc.sync.dma_start(out=c16_sb, in_=c16[:])
        # w2_sb[p, h, m] = W2[h*128+m, p]
        w2_sb = c16_sb[:, 0:256].rearrange("p (h m) -> p h m", h=2)

        xt0 = xpool.tile([128, 2, 2, 2, TW], F8, name="xt")
        nc.sync.dma_start(out=xt0[:, 0], in_=xmr[0, :, 0])
        nc.sync.dma_start(out=xt0[:, 1], in_=xmr[0, :, 1])

        xt1 = xpool.tile([128, 2, 2, 2, TW], F8, name="xt")
        nc.sync.dma_start(out=xt1, in_=xmr[1])

        nc.vector.memset(a_buf[:, 0:1], 0.0)
        nc.vector.memset(b_buf[:, 0:1], 0.0)

        # DVFS warmup + ACT table preload: the PE starts in its lowest
        # p-state and ramps only while busy; the first Sigmoid pays a
        # 1283ns table load.  Both are hidden in the ~10us prologue dead
        # time (preamble + first DMAs).
        scratch = singles.tile([128, 2, TW], F8)
        nc.vector.memset(scratch, 0.0)
        warm_in = singles.tile([128, 1], F16)
        nc.gpsimd.memset(warm_in, 0.0)
        warm_act = singles.tile([128, 1], F16)
        nc.scalar.activation(warm_act, warm_in, SIG)
        for w in range(4):
            ow = (psO0 if w % 2 == 0 else psO1).tile([128, 2, TW], F32, name="o")
            for t in range(2 if w else 1):
                nc.tensor.matmul(ow[:, t, :], lhsT=scratch[:, :, 0:128],
                                 rhs=scratch, start=True, stop=True,
                                 perf_mode=DR)

        # Widen f16 biases to f32 once (ACT bias/scale operands read f32).
        bias_sb = singles.tile([128, 8], F32)
        nc.vector.tensor_scalar(out=bias_sb, in0=c16_sb[:, 256:264],
                                scalar1=1.0, scalar2=None, op0=MUL)

        def phase_c(q, pools=None, names=("o", "o")):
            # output GEMM + activation + store for pair q (z cols
            # [HALO+1024q, HALO+1024q+1024), out cols [1024q, 1024q+1024))
            zc = HALO + 1024 * q
            if pools is None:
                pools = (psO0, psO1)
            for h, pool in ((0, pools[0]), (1, pools[1])):
                o = pool.tile([128, 2, TW], F32, name=names[h])
                for t in range(2):
                    nc.tensor.matmul(o[:, t, :], lhsT=w2_sb[:, h, :],
                                     rhs=z_buf[:, zc + TW * t:zc + TW * (t + 1)],
                                     start=True, stop=True)
                ot = opool.tile([128, 1024], F16, name="ot")
                nc.scalar.activation(ot, o[:].rearrange("p t w -> p (t w)"),
                                     SIG, bias=bias_sb[:, 2 + h:3 + h],
                                     scale=-1.0)
                nc.sync.dma_start(out=outr[q, :, h, :], in_=ot)

        # ---- halo tile (columns [0, HALO)) ----
        gh = [psG0.tile([128, 2, TW], F32, name="g"),
              psG1.tile([128, 2, TW], F32, name="g")]
        for h in range(2):
            for c in range(2):
                nc.tensor.matmul(gh[h][:, 0, 0:HALO], lhsT=w1_sb[:, c, :, h, :],
                                 rhs=xh_sb[:, c, :, :],
                                 start=(c == 0), stop=(c == 1), perf_mode=DR)
        lrh = hpool.tile([128, 2, HALO], F16)
        for h in range(2):
            nc.scalar.activation(lrh[:, h, :], gh[h][:, 0, 0:HALO], SIG,
                                 bias=bias_sb[:, h:h + 1], scale=1.0 / W1_SCALE)
        nc.gpsimd.tensor_scalar(out=b_buf[:, 1:1 + HALO],
                                in0=lrh[:, 0, :],
                                scalar1=-1.0, scalar2=1.0, op0=MUL, op1=ADD)
        nc.vector.tensor_tensor(out=a_buf[:, 1:1 + HALO],
                                in0=lrh[:, 0, :],
                                in1=lrh[:, 1, :], op=MUL)
        # Halo b *= bscale (0 on core 0 so the state stays exactly 0)
        nc.vector.tensor_scalar(out=b_buf[:, 0:HALO + 1],
                                in0=b_buf[:, 0:HALO + 1],
                                scalar1=bias_sb[:, 4:5], scalar2=None, op0=MUL)
        nc.vector.tensor_tensor_scan(out=z_buf[:, 0:HALO],
                                     data0=a_buf[:, 0:HALO],
                                     data1=b_buf[:, 0:HALO],
                                     initial=0.0, op0=MUL, op1=ADD)

        DELAY = 2  # pairs of lead distance between phase A/B and phase C

        # ---- owned pairs ----
        for p in range(NPAIR):
            c0 = HALO + 1024 * p
            if p == 0:
                xt = xt0
            elif p == 1:
                xt = xt1
            else:
                xt = xpool.tile([128, 2, 2, 2, TW], F8, name="xt")
                nc.sync.dma_start(out=xt, in_=xmr[p])
            lr = hpool.tile([128, 2, 1024], F16)
            if p <= 1:
                # Pipeline fill: each (h, t) quarter gets its OWN PSUM tile
                # (borrowing the psO rings, idle until phase_c(0) at p=2)
                # so each 512-col act fires right after its two matmuls --
                # tile deps are whole-tile, so sharing a tile would delay
                # the first sigmoid by ~1.7us.  Pairs 0-1 run while the PE
                # is still at mid-DVFS, so earlier acts fill ramp gaps.
                for h, t, pool, nm in ((0, 0, psG0, "g"), (0, 1, psO0, "o"),
                                       (1, 0, psG1, "g"), (1, 1, psO1, "o")):
                    g = pool.tile([128, 2, TW], F32, name=nm)
                    for c in range(2):
                        nc.tensor.matmul(
                            g[:, 0, :], lhsT=w1_sb[:, c, :, h, :],
                            rhs=xt[:, t, c, :, :],
                            start=(c == 0), stop=(c == 1), perf_mode=DR)
                    nc.scalar.activation(lr[:, h, TW * t:TW * (t + 1)],
                                         g[:, 0, :], SIG,
                                         bias=bias_sb[:, h:h + 1],
                                         scale=1.0 / W1_SCALE)
            else:
                # h-major GEMM order: ACT-h0 fires after the first two
                # groups while h1 still computes; per-h PSUM pools stagger
                # the WARs.
                for h, pool in ((0, psG0), (1, psG1)):
                    g = pool.tile([128, 2, TW], F32, name="g")
                    for t in range(2):
                        for c in range(2):
                            nc.tensor.matmul(
                                g[:, t, :], lhsT=w1_sb[:, c, :, h, :],
                                rhs=xt[:, t, c, :, :],
                                start=(c == 0), stop=(c == 1), perf_mode=DR)
                    nc.scalar.activation(lr[:, h, :],
                                         g[:].rearrange("p t w -> p (t w)"),
                                         SIG, bias=bias_sb[:, h:h + 1],
                                         scale=1.0 / W1_SCALE)
            nc.gpsimd.tensor_scalar(out=b_buf[:, c0 + 1:c0 + 1025],
                                    in0=lr[:, 0, :],
                                    scalar1=-1.0, scalar2=1.0,
                                    op0=MUL, op1=ADD)
            if p < NPAIR - 1:
                nc.vector.tensor_tensor(out=a_buf[:, c0 + 1:c0 + 1025],
                                        in0=lr[:, 0, :], in1=lr[:, 1, :],
                                        op=MUL)
                nc.vector.tensor_tensor_scan(out=z_buf[:, c0:c0 + 1024],
                                             data0=a_buf[:, c0:c0 + 1024],
                                             data1=b_buf[:, c0:c0 + 1024],
                                             initial=z_buf[:, c0 - 1:c0],
                                             op0=MUL, op1=ADD)
                if p - DELAY >= 0:
                    phase_c(p - DELAY)
            else:
                # Final pair: split a/scan into halves so the tail output
                # chain starts half a pair earlier.
                nc.vector.tensor_tensor(out=a_buf[:, c0 + 1:c0 + TW + 1],
                                        in0=lr[:, 0, 0:TW], in1=lr[:, 1, 0:TW],
                                        op=MUL)
                nc.vector.tensor_tensor_scan(out=z_buf[:, c0:c0 + TW],
                                             data0=a_buf[:, c0:c0 + TW],
                                             data1=b_buf[:, c0:c0 + TW],
                                             initial=z_buf[:, c0 - 1:c0],
                                             op0=MUL, op1=ADD)
                nc.vector.tensor_tensor(out=a_buf[:, c0 + TW + 1:c0 + 1025],
                                        in0=lr[:, 0, TW:1024],
                                        in1=lr[:, 1, TW:1024], op=MUL)
                phase_c(p - DELAY)
                nc.vector.tensor_tensor_scan(
                    out=z_buf[:, c0 + TW:c0 + 1024],
                    data0=a_buf[:, c0 + TW:c0 + 1024],
                    data1=b_buf[:, c0 + TW:c0 + 1024],
                    initial=z_buf[:, c0 + TW - 1:c0 + TW],
                    op0=MUL, op1=ADD)
                # Pair 6 out-GEMMs run on the psG rings (free right after
                # pair 7's h-acts) instead of waiting for pair-5's
                # out-acts to release the psO banks.
                phase_c(p - 1, pools=(psG0, psG1), names=("g", "g"))
                # Pair 7 out-GEMMs on the psO rings (free after out(5));
                # 512-col acts + 128KB stores so each quarter drains as
                # soon as its scan half + GEMM are done.
                of = [psO0.tile([128, 2, TW], F32, name="o"),
                      psO1.tile([128, 2, TW], F32, name="o")]
                otf = [opool.tile([128, 1024], F16, name="ot"),
                       opool.tile([128, 1024], F16, name="ot")]
                outq = out[:].rearrange("q p (h t w) -> q p h t w", h=2, t=2)
                for t in range(2):
                    for h in range(2):
                        nc.tensor.matmul(
                            of[h][:, t, :], lhsT=w2_sb[:, h, :],
                            rhs=z_buf[:, c0 + TW * t:c0 + TW * (t + 1)],
                            start=True, stop=True)
                for t in range(2):
                    for h in range(2):
                        nc.scalar.activation(otf[h][:, TW * t:TW * (t + 1)],
                                             of[h][:, t, :], SIG,
                                             bias=bias_sb[:, 2 + h:3 + h],
                                             scale=-1.0)
                        nc.sync.dma_start(out=outq[p, :, h, t, :],
                                          in_=otf[h][:, TW * t:TW * (t + 1)])

    nc.finalize()
    return nc


def _make_in_maps(inputs, W1, B1, W2, B2):
    inputs = np.asarray(inputs, dtype=np.float32)
    W1 = np.asarray(W1, dtype=np.float32)
    B1 = np.asarray(B1, dtype=np.float32)
    W2 = np.asarray(W2, dtype=np.float32)
    B2 = np.asarray(B2, dtype=np.float32)

    x8 = inputs.astype(F8_NP)
    # w1[p, c, i, h, m] = 8*W1[h*128+m, c*256+i*128+p]
    w1p = np.ascontiguousarray(
        (W1 * W1_SCALE).astype(F8_NP)
        .reshape(2, 128, 2, 2, 128)            # h, m, c, i, p
        .transpose(4, 2, 3, 0, 1)              # p, c, i, h, m
        .reshape(128, 1024))
    # w2[p, h, m] = W2[h*128+m, p]
    w2p = np.ascontiguousarray(
        W2.astype(F16_NP).reshape(2, 128, 128)  # h, m, p
        .transpose(2, 0, 1).reshape(128, 256))
    biasc = np.zeros((128, 8), np.float16)
    biasc[:, 0] = B1[:128, 0].astype(np.float16)
    biasc[:, 1] = B1[128:, 0].astype(np.float16)
    biasc[:, 2] = (-B2[:128, 0]).astype(np.float16)
    biasc[:, 3] = (-B2[128:, 0]).astype(np.float16)

    in_maps = []
    for i in range(N_CORES):
        s = i * T_LOC
        lo = s - HALO
        if lo < 0:
            xs = np.concatenate(
                [np.zeros((INPUT_LEN, -lo), F8_NP), x8[:, :s + T_LOC]], axis=1)
        else:
            xs = x8[:, lo:s + T_LOC]
        xr = xs.reshape(2, 2, 128, W_IN)                  # c, i, p, col
        xhm = np.ascontiguousarray(
            xr[:, :, :, :HALO].transpose(2, 0, 1, 3).reshape(128, 4 * HALO))
        xm = np.ascontiguousarray(
            xr[:, :, :, HALO:].reshape(2, 2, 128, NPAIR, 2, TW)
            .transpose(3, 2, 4, 0, 1, 5)                  # pair, p, t, c, i, w
            .reshape(NPAIR, 128, 4096))
        b = biasc.copy()
        b[:, 4] = 0.0 if i == 0 else 1.0
        c8p = np.ascontiguousarray(np.concatenate([w1p, xhm], axis=1))
        c16p = np.ascontiguousarray(
            np.concatenate([w2p, b.astype(F16_NP)], axis=1))
        in_maps.append({
            "c8": c8p, "c16": c16p, "x_main": xm,
        })
    return in_maps


def _run(inputs, W1, B1, W2, B2, trace=False, **kw):
    from concourse.bass_utils import run_bass_kernel_spmd

    nc = _build_nc()
    in_maps = _make_in_maps(inputs, W1, B1, W2, B2)
    res = run_bass_kernel_spmd(nc, in_maps, list(range(N_CORES)), trace=trace, **kw)
    parts = []
    for r in res.results:
        o = np.asarray(r["out"]).astype(np.float32)       # (NPAIR, 128, 2048)
        o = (o.reshape(NPAIR, 128, 2, 2, TW)              # pair, p, h, t, w
             .transpose(2, 1, 0, 3, 4)                    # h, p, pair, t, w
             .reshape(OUT_LEN, T_LOC))
        parts.append(o)
    full = np.concatenate(parts, axis=1)
    return full, res


def kernel(inputs, W1, B1, W2, B2):
    full, _ = _run(inputs, W1, B1, W2, B2, trace=False)
    return full.astype(np.float32, copy=False)


# revision 45
# speedup vs baseline: 1.1729x; 1.1729x over previous
"""Deductron kernel for Trainium2, 8 NeuronCores, time-sharded.

Math (matching the reference):
    h = sigmoid(W1 @ x + B1); left, right = h[:128], h[128:]
    a_t = left_t * right_t; b_t = 1 - left_t
    u_0 = 0; u_t = a_{t-1} * u_{t-1} + b_{t-1}   (z[:, t] = u_t)
    out = 1 - sigmoid(W2 @ z + B2) = sigmoid(-(W2 @ z + B2))

Sharding: the 65536-frame time axis is split into 8 chunks of 8192 plus a
128-frame left washout halo per core (measured worst-case prod(a) over any
128-step boundary window is ~e^-182, so the recurrence state forgets its
initial condition well inside the halo; core 0's halo input is zero-padded
and its halo b is scaled by 0 so the state stays exactly 0).  HALO=32 was
tried and reverted: it intermittently triggered a ~10us slower execution
mode (likely SBUF alignment of the 1024-col block starts).

Key implementation points (v2 -- ACT-batched; 61.7us -> ~55us):
  * GEMM1 runs in fp8e4 (e4m3) with MatmulPerfMode.DoubleRow; W1 pre-scaled
    by 8 on host (avoids fp8 subnormals); the h-activation applies scale=1/8.
  * The Activation engine is the bottleneck (1 elem/cycle/lane @1.2GHz plus
    ~150ns fixed per instruction).  PSUM is laid out h-major -- four 2-bank
    pools psG0/psG1/psO0/psO1, one per distinct sigmoid bias -- so every
    sigmoid instruction spans 1024 columns (4 instrs/pair instead of 8),
    cutting ACT to ~4.0us/pair (measured back-to-back through the steady
    state).  Per-h pools keep GEMM(p+1) pipelined behind ACT-h(p) despite
    bufs=1 (tile deps are whole-tile).
  * Prologue: constants travel in TWO DMAs (fp8: W1|halo-x, f16: W2|biases,
    widened to f32 once on DVE); pair-0's input is split into two
    half-loads, and pairs 0-1 each use four borrowed single-t PSUM tiles
    (psO rings are idle until phase_c(0) at p=2) so 512-col sigmoids fire
    per 2-matmul group during the mid-DVFS ramp; a dummy 1-col sigmoid
    preloads the 1283ns activation table during the preamble; 7 throwaway
    matmuls climb the PE's DVFS ladder (full clock needs ~9us of
    sustained activity).
  * b = 1-left on GpSimd (a on GpSimd measured 2-4us -- Q7 tensor_tensor
    is far below roofline); a = left*right and the recurrence scan
    (tensor_tensor_scan, fp32 state, ~2.3ns/col on HW) on DVE.
  * Frames processed in PAIRS of 512-col tiles: one contiguous 512KB input
    DMA and one 512KB store per pair (DMA issue costs ~600ns serial on the
    issuing queue; completion semaphores add 900ns).
  * Drain: pair-6's out-GEMMs run on the psG rings (free after the last
    h-acts) and pair-7's on the psO rings (free after out-5), avoiding
    psO WAR stalls and scheduler head-of-line blocking; pair-7's a/scan
    are split in halves and its out-acts/stores go per 512-col quarter.
"""

import sys

for _p in ("/opt/trn_rl_repo", "/opt/pypackages"):
    if _p not in sys.path:
        sys.path.append(_p)

import numpy as np
import ml_dtypes

# Problem constants (hardcoded per contract).
INPUT_LEN = 512
N_MEM = 128
OUT_LEN = 256
T_TOTAL = 65536
N_CORES = 8
T_LOC = T_TOTAL // N_CORES   # 8192 owned frames per core
HALO = 64                    # washout halo (see module docstring)
TW = 512                     # column tile width (one PSUM bank of fp32)
NPAIR = T_LOC // (2 * TW)    # 8 pairs of owned tiles per core
W_IN = HALO + T_LOC          # 8256
W1_SCALE = 8.0               # host multiplies W1 by this; ACT applies 1/8

F16_NP = np.float16
F8_NP = ml_dtypes.float8_e4m3fn


def _build_nc():
    import concourse.tile as tile
    from concourse import bacc, mybir
    from contextlib import ExitStack

    F32 = mybir.dt.float32
    F16 = mybir.dt.float16
    F8 = mybir.dt.float8e4
    SIG = mybir.ActivationFunctionType.Sigmoid
    MUL = mybir.AluOpType.mult
    ADD = mybir.AluOpType.add
    DR = mybir.MatmulPerfMode.DoubleRow

    nc = bacc.Bacc()
    # DRAM layouts are host-packed so every DMA is fully contiguous.
    # c8[:, 0:1024] = w1 packed; c8[:, 1024:1280] = halo x packed.
    c8 = nc.dram_tensor("c8", [128, 1024 + 4 * HALO], F8, kind="ExternalInput")
    # c16[:, 0:256] = w2 packed; c16[:, 256:264] = biases
    # [B1a, B1b, -B2a, -B2b, bscale, 0, 0, 0].
    c16 = nc.dram_tensor("c16", [128, 264], F16, kind="ExternalInput")
    x_main = nc.dram_tensor("x_main", [NPAIR, 128, 4096], F8, kind="ExternalInput")
    out = nc.dram_tensor("out", [NPAIR, 128, 2048], F16, kind="ExternalOutput")

    with ExitStack() as ctx:
        tc = ctx.enter_context(tile.TileContext(nc))
        singles = ctx.enter_context(tc.tile_pool(name="singles", bufs=1))
        xpool = ctx.enter_context(tc.tile_pool(name="xpool", bufs=4))
        hpool = ctx.enter_context(tc.tile_pool(name="hpool", bufs=4))
        opool = ctx.enter_context(tc.tile_pool(name="opool", bufs=6))
        # One 2-bank PSUM pool per distinct sigmoid bias: h-GEMM halves
        # (B1a/B1b) and out-GEMM halves (-B2a/-B2b).  Each tile holds both
        # 512-col t-tiles of a pair adjacently -> 1024-col ACT instructions.
        psG0 = ctx.enter_context(tc.tile_pool(name="psG0", bufs=1, space="PSUM"))
        psG1 = ctx.enter_context(tc.tile_pool(name="psG1", bufs=1, space="PSUM"))
        psO0 = ctx.enter_context(tc.tile_pool(name="psO0", bufs=1, space="PSUM"))
        psO1 = ctx.enter_context(tc.tile_pool(name="psO1", bufs=1, space="PSUM"))

        # Persistent recurrence buffers. a_buf/b_buf are written at a +1
        # column offset (a_buf[:, p] = a at input column p-1) so the scan
        # output z[:, p] = u at column p directly.
        a_buf = singles.tile([N_MEM, W_IN + 1], F16)
        b_buf = singles.tile([N_MEM, W_IN + 1], F16)
        z_buf = singles.tile([N_MEM, W_IN], F16)

        # ---- constants: two packed DMAs; pair-0 input is issue #2 so its
        # data lands as early as possible (each dma_start serializes ~600ns
        # on the Sync queue).
        c8_sb = singles.tile([128, 1024 + 4 * HALO], F8)
        nc.sync.dma_start(out=c8_sb, in_=c8[:])
        # w1_sb[p, c, i, h, m] = 8*W1[h*128+m, c*256 + i*128 + p]
        w1_sb = c8_sb[:, 0:1024].rearrange("p (c i h m) -> p c i h m", c=2, i=2, h=2)
        xh_sb = c8_sb[:, 1024:1024 + 4 * HALO].rearrange(
            "p (c i w) -> p c i w", c=2, i=2)

        xmr = x_main[:].rearrange("q p (t c i w) -> q p t c i w", t=2, c=2, i=2)
        outr = out[:].rearrange("q p (h w) -> q p h w", h=2)

        # Biases ride the small c16 DMA -- issue #2 so the halo activations
        # aren't bias-gated.  Pair 0 arrives as two half-loads so its first
        # 512-col GEMM (and sigmoid) starts before the full pair lands.
        c16_sb = singles.tile([128, 264], F16)
        nc.sync.dma_start(out=c16_sb, in_=c16[:])
        # w2_sb[p, h, m] = W2[h*128+m, p]
        w2_sb = c16_sb[:, 0:256].rearrange("p (h m) -> p h m", h=2)

        xt0 = xpool.tile([128, 2, 2, 2, TW], F8, name="xt")
        nc.sync.dma_start(out=xt0[:, 0], in_=xmr[0, :, 0])
        nc.sync.dma_start(out=xt0[:, 1], in_=xmr[0, :, 1])

        xt1 = xpool.tile([128, 2, 2, 2, TW], F8, name="xt")
        nc.sync.dma_start(out=xt1, in_=xmr[1])

        nc.vector.memset(a_buf[:, 0:1], 0.0)
        nc.vector.memset(b_buf[:, 0:1], 0.0)

        # DVFS warmup + ACT table preload: the PE starts in its lowest
        # p-state and ramps only while busy; the first Sigmoid pays a
        # 1283ns table load.  Both are hidden in the ~10us prologue dead
        # time (preamble + first DMAs).
        scratch = singles.tile([128, 2, TW], F8)
        nc.vector.memset(scratch, 0.0)
        warm_in = singles.tile([128, 1], F16)
        nc.gpsimd.memset(warm_in, 0.0)
        warm_act = singles.tile([128, 1], F16)
        nc.scalar.activation(warm_act, warm_in, SIG)
        for w in range(4):
            ow = (psO0 if w % 2 == 0 else psO1).tile([128, 2, TW], F32, name="o")
            for t in range(2 if w else 1):
                nc.tensor.matmul(ow[:, t, :], lhsT=scratch[:, :, 0:128],
                                 rhs=scratch, start=True, stop=True,
                                 perf_mode=DR)

        # Widen f16 biases to f32 once (ACT bias/scale operands read f32).
        bias_sb = singles.tile([128, 8], F32)
        nc.vector.tensor_scalar(out=bias_sb, in0=c16_sb[:, 256:264],
                                scalar1=1.0, scalar2=None, op0=MUL)

        def phase_c(q, pools=None, names=("o", "o")):
            # output GEMM + activation + store for pair q (z cols
            # [HALO+1024q, HALO+1024q+1024), out cols [1024q, 1024q+1024))
            zc = HALO + 1024 * q
            if pools is None:
                pools = (psO0, psO1)
            for h, pool in ((0, pools[0]), (1, pools[1])):
                o = pool.tile([128, 2, TW], F32, name=names[h])
                for t in range(2):
                    nc.tensor.matmul(o[:, t, :], lhsT=w2_sb[:, h, :],
                                     rhs=z_buf[:, zc + TW * t:zc + TW * (t + 1)],
                                     start=True, stop=True)
                ot = opool.tile([128, 1024], F16, name="ot")
                nc.scalar.activation(ot, o[:].rearrange("p t w -> p (t w)"),
                                     SIG, bias=bias_sb[:, 2 + h:3 + h],
                                     scale=-1.0)
                nc.sync.dma_start(out=outr[q, :, h, :], in_=ot)

        # ---- halo tile (columns [0, HALO)) ----
        gh = [psG0.tile([128, 2, TW], F32, name="g"),
              psG1.tile([128, 2, TW], F32, name="g")]
        for h in range(2):
            for c in range(2):
                nc.tensor.matmul(gh[h][:, 0, 0:HALO], lhsT=w1_sb[:, c, :, h, :],
                                 rhs=xh_sb[:, c, :, :],
                                 start=(c == 0), stop=(c == 1), perf_mode=DR)
        lrh = hpool.tile([128, 2, HALO], F16)
        for h in range(2):
            nc.scalar.activation(lrh[:, h, :], gh[h][:, 0, 0:HALO], SIG,
                                 bias=bias_sb[:, h:h + 1], scale=1.0 / W1_SCALE)
        nc.gpsimd.tensor_scalar(out=b_buf[:, 1:1 + HALO],
                                in0=lrh[:, 0, :],
                                scalar1=-1.0, scalar2=1.0, op0=MUL, op1=ADD)
        nc.vector.tensor_tensor(out=a_buf[:, 1:1 + HALO],
                                in0=lrh[:, 0, :],
                                in1=lrh[:, 1, :], op=MUL)
        # Halo b *= bscale (0 on core 0 so the state stays exactly 0)
        nc.vector.tensor_scalar(out=b_buf[:, 0:HALO + 1],
                                in0=b_buf[:, 0:HALO + 1],
                                scalar1=bias_sb[:, 4:5], scalar2=None, op0=MUL)
        nc.vector.tensor_tensor_scan(out=z_buf[:, 0:HALO],
                                     data0=a_buf[:, 0:HALO],
                                     data1=b_buf[:, 0:HALO],
                                     initial=0.0, op0=MUL, op1=ADD)

        DELAY = 2  # pairs of lead distance between phase A/B and phase C

        # ---- owned pairs ----
        for p in range(NPAIR):
            c0 = HALO + 1024 * p
            if p == 0:
                xt = xt0
            elif p == 1:
                xt = xt1
            else:
                xt = xpool.tile([128, 2, 2, 2, TW], F8, name="xt")
                nc.sync.dma_start(out=xt, in_=xmr[p])
            lr = hpool.tile([128, 2, 1024], F16)
            if p <= 1:
                # Pipeline fill: each (h, t) quarter gets its OWN PSUM tile
                # (borrowing the psO rings, idle until phase_c(0) at p=2)
                # so each 512-col act fires right after its two matmuls --
                # tile deps are whole-tile, so sharing a tile would delay
                # the first sigmoid by ~1.7us.  Pairs 0-1 run while the PE
                # is still at mid-DVFS, so earlier acts fill ramp gaps.
                for h, t, pool, nm in ((0, 0, psG0, "g"), (0, 1, psO0, "o"),
                                       (1, 0, psG1, "g"), (1, 1, psO1, "o")):
                    g = pool.tile([128, 2, TW], F32, name=nm)
                    for c in range(2):
                        nc.tensor.matmul(
                            g[:, 0, :], lhsT=w1_sb[:, c, :, h, :],
                            rhs=xt[:, t, c, :, :],
                            start=(c == 0), stop=(c == 1), perf_mode=DR)
                    nc.scalar.activation(lr[:, h, TW * t:TW * (t + 1)],
                                         g[:, 0, :], SIG,
                                         bias=bias_sb[:, h:h + 1],
                                         scale=1.0 / W1_SCALE)
            else:
                # h-major GEMM order: ACT-h0 fires after the first two
                # groups while h1 still computes; per-h PSUM pools stagger
                # the WARs.
                for h, pool in ((0, psG0), (1, psG1)):
                    g = pool.tile([128, 2, TW], F32, name="g")
                    for t in range(2):
                        for c in range(2):
                            nc.tensor.matmul(
                                g[:, t, :], lhsT=w1_sb[:, c, :, h, :],
                                rhs=xt[:, t, c, :, :],
                                start=(c == 0), stop=(c == 1), perf_mode=DR)
                    nc.scalar.activation(lr[:, h, :],
                                         g[:].rearrange("p t w -> p (t w)"),
                                         SIG, bias=bias_sb[:, h:h + 1],
                                         scale=1.0 / W1_SCALE)
            nc.gpsimd.tensor_scalar(out=b_buf[:, c0 + 1:c0 + 1025],
                                    in0=lr[:, 0, :],
                                    scalar1=-1.0, scalar2=1.0,
                                    op0=MUL, op1=ADD)
            if p < NPAIR - 1:
                nc.vector.tensor_tensor(out=a_buf[:, c0 + 1:c0 + 1025],
                                        in0=lr[:, 0, :], in1=lr[:, 1, :],
                                        op=MUL)
                nc.vector.tensor_tensor_scan(out=z_buf[:, c0:c0 + 1024],
                                             data0=a_buf[:, c0:c0 + 1024],
                                             data1=b_buf[:, c0:c0 + 1024],
                                             initial=z_buf[:, c0 - 1:c0],
                                             op0=MUL, op1=ADD)
                if p - DELAY >= 0:
                    phase_c(p - DELAY)
            else:
                # Final pair: split a/scan into halves so the tail output
                # chain starts half a pair earlier.
                nc.vector.tensor_tensor(out=a_buf[:, c0 + 1:c0 + TW + 1],
                                        in0=lr[:, 0, 0:TW], in1=lr[:, 1, 0:TW],
                                        op=MUL)
                nc.vector.tensor_tensor_scan(out=z_buf[:, c0:c0 + TW],
                                             data0=a_buf[:, c0:c0 + TW],
                                             data1=b_buf[:, c0:c0 + TW],
                                             initial=z_buf[:, c0 - 1:c0],
                                             op0=MUL, op1=ADD)
                nc.vector.tensor_tensor(out=a_buf[:, c0 + TW + 1:c0 + 1025],
                                        in0=lr[:, 0, TW:1024],
                                        in1=lr[:, 1, TW:1024], op=MUL)
                phase_c(p - DELAY)
                nc.vector.tensor_tensor_scan(
                    out=z_buf[:, c0 + TW:c0 + 1024],
                    data0=a_buf[:, c0 + TW:c0 + 1024],
                    data1=b_buf[:, c0 + TW:c0 + 1024],
                    initial=z_buf[:, c0 + TW - 1:c0 + TW],
                    op0=MUL, op1=ADD)
                # Pair 6 out-GEMMs run on the psG rings (free right after
                # pair 7's h-acts) instead of waiting for pair-5's
                # out-acts to release the psO banks.
                phase_c(p - 1, pools=(psG0, psG1), names=("g", "g"))
                # Pair 7 out-GEMMs on the psO rings (free after out(5));
                # 512-col acts + 128KB stores so each quarter drains as
                # soon as its scan half + GEMM are done.
                of = [psO0.tile([128, 2, TW], F32, name="o"),
                      psO1.tile([128, 2, TW], F32, name="o")]
                otf = [opool.tile([128, 1024], F16, name="ot"),
                       opool.tile([128, 1024], F16, name="ot")]
                outq = out[:].rearrange("q p (h t w) -> q p h t w", h=2, t=2)
                for t in range(2):
                    for h in range(2):
                        nc.tensor.matmul(
                            of[h][:, t, :], lhsT=w2_sb[:, h, :],
                            rhs=z_buf[:, c0 + TW * t:c0 + TW * (t + 1)],
                            start=True, stop=True)
                for t in range(2):
                    for h in range(2):
                        nc.scalar.activation(otf[h][:, TW * t:TW * (t + 1)],
                                             of[h][:, t, :], SIG,
                                             bias=bias_sb[:, 2 + h:3 + h],
                                             scale=-1.0)
                        nc.sync.dma_start(out=outq[p, :, h, t, :],
                                          in_=otf[h][:, TW * t:TW * (t + 1)])

    nc.finalize()
    return nc


def _make_in_maps(inputs, W1, B1, W2, B2):
    inputs = np.asarray(inputs, dtype=np.float32)
    W1 = np.asarray(W1, dtype=np.float32)
    B1 = np.asarray(B1, dtype=np.float32)
    W2 = np.asarray(W2, dtype=np.float32)
    B2 = np.asarray(B2, dtype=np.float32)

    x8 = inputs.astype(F8_NP)
    # w1[p, c, i, h, m] = 8*W1[h*128+m, c*256+i*128+p]
    w1p = np.ascontiguousarray(
        (W1 * W1_SCALE).astype(F8_NP)
        .reshape(2, 128, 2, 2, 128)            # h, m, c, i, p
        .transpose(4, 2, 3, 0, 1)              # p, c, i, h, m
        .reshape(128, 1024))
    # w2[p, h, m] = W2[h*128+m, p]
    w2p = np.ascontiguousarray(
        W2.astype(F16_NP).reshape(2, 128, 128)  # h, m, p
        .transpose(2, 0, 1).reshape(128, 256))
    biasc = np.zeros((128, 8), np.float16)
    biasc[:, 0] = B1[:128, 0].astype(np.float16)
    biasc[:, 1] = B1[128:, 0].astype(np.float16)
    biasc[:, 2] = (-B2[:128, 0]).astype(np.float16)
    biasc[:, 3] = (-B2[128:, 0]).astype(np.float16)

    in_maps = []
    for i in range(N_CORES):
        s = i * T_LOC
        lo = s - HALO
        if lo < 0:
            xs = np.concatenate(
                [np.zeros((INPUT_LEN, -lo), F8_NP), x8[:, :s + T_LOC]], axis=1)
        else:
            xs = x8[:, lo:s + T_LOC]
        xr = xs.reshape(2, 2, 128, W_IN)                  # c, i, p, col
        xhm = np.ascontiguousarray(
            xr[:, :, :, :HALO].transpose(2, 0, 1, 3).reshape(128, 4 * HALO))
        xm = np.ascontiguousarray(
            xr[:, :, :, HALO:].reshape(2, 2, 128, NPAIR, 2, TW)
            .transpose(3, 2, 4, 0, 1, 5)                  # pair, p, t, c, i, w
            .reshape(NPAIR, 128, 4096))
        b = biasc.copy()
        b[:, 4] = 0.0 if i == 0 else 1.0
        c8p = np.ascontiguousarray(np.concatenate([w1p, xhm], axis=1))
        c16p = np.ascontiguousarray(
            np.concatenate([w2p, b.astype(F16_NP)], axis=1))
        in_maps.append({
            "c8": c8p, "c16": c16p, "x_main": xm,
        })
    return in_maps


def _run(inputs, W1, B1, W2, B2, trace=False, **kw):
    from concourse.bass_utils import run_bass_kernel_spmd

    nc = _build_nc()
    in_maps = _make_in_maps(inputs, W1, B1, W2, B2)
    res = run_bass_kernel_spmd(nc, in_maps, list(range(N_CORES)), trace=trace, **kw)
    parts = []
    for r in res.results:
        o = np.asarray(r["out"]).astype(np.float32)       # (NPAIR, 128, 2048)
        o = (o.reshape(NPAIR, 128, 2, 2, TW)              # pair, p, h, t, w
             .transpose(2, 1, 0, 3, 4)                    # h, p, pair, t, w
             .reshape(OUT_LEN, T_LOC))
        parts.append(o)
    full = np.concatenate(parts, axis=1)
    return full, res


def kernel(inputs, W1, B1, W2, B2):
    full, _ = _run(inputs, W1, B1, W2, B2, trace=False)
    return full.astype(np.float32, copy=False)


# revision 46
# speedup vs baseline: 1.1901x; 1.0146x over previous
"""Deductron kernel for Trainium2, 8 NeuronCores, time-sharded.

Math (matching the reference):
    h = sigmoid(W1 @ x + B1); left, right = h[:128], h[128:]
    a_t = left_t * right_t; b_t = 1 - left_t
    u_0 = 0; u_t = a_{t-1} * u_{t-1} + b_{t-1}   (z[:, t] = u_t)
    out = 1 - sigmoid(W2 @ z + B2) = sigmoid(-(W2 @ z + B2))

Sharding: the 65536-frame time axis is split into 8 chunks of 8192 plus a
64-frame left washout halo per core (measured worst-case prod(a) over any
64-step boundary window is ~e^-91, so the recurrence state forgets its
initial condition well inside the halo; core 0's halo input is zero-padded
and its halo b is scaled by 0 so the state stays exactly 0).  HALO of 32/
64/128 are statistically tied; a rare ~63us slow-execution mode seen under
all of them is machine-state, not a kernel property.

Key implementation points (v2 -- ACT-batched; 61.7us -> ~55us):
  * GEMM1 runs in fp8e4 (e4m3) with MatmulPerfMode.DoubleRow; W1 pre-scaled
    by 8 on host (avoids fp8 subnormals); the h-activation applies scale=1/8.
  * The Activation engine is the bottleneck (1 elem/cycle/lane @1.2GHz plus
    ~150ns fixed per instruction).  PSUM is laid out h-major -- four 2-bank
    pools psG0/psG1/psO0/psO1, one per distinct sigmoid bias -- so every
    sigmoid instruction spans 1024 columns (4 instrs/pair instead of 8),
    cutting ACT to ~4.0us/pair (measured back-to-back through the steady
    state).  Per-h pools keep GEMM(p+1) pipelined behind ACT-h(p) despite
    bufs=1 (tile deps are whole-tile).
  * Prologue: constants travel in TWO DMAs (fp8: W1|halo-x, f16: W2|biases,
    widened to f32 once on DVE); pair-0's input is split into two
    half-loads, and pairs 0-1 each use four borrowed single-t PSUM tiles
    (psO rings are idle until phase_c(0) at p=2) so 512-col sigmoids fire
    per 2-matmul group during the mid-DVFS ramp; a dummy 1-col sigmoid
    preloads the 1283ns activation table during the preamble; 7 throwaway
    matmuls climb the PE's DVFS ladder (full clock needs ~9us of
    sustained activity).
  * b = 1-left on GpSimd (a on GpSimd measured 2-4us -- Q7 tensor_tensor
    is far below roofline); a = left*right and the recurrence scan
    (tensor_tensor_scan, fp32 state, ~2.3ns/col on HW) on DVE.
  * Frames processed in PAIRS of 512-col tiles: one contiguous 512KB input
    DMA and one 512KB store per pair (DMA issue costs ~600ns serial on the
    issuing queue; completion semaphores add 900ns).
  * Drain: pair-6's out-GEMMs run on the psG rings (free after the last
    h-acts) and pair-7's on the psO rings (free after out-5), avoiding
    psO WAR stalls and scheduler head-of-line blocking; pair-7's a/scan
    are split in halves and its out-acts/stores go per 512-col quarter.
"""

import sys

for _p in ("/opt/trn_rl_repo", "/opt/pypackages"):
    if _p not in sys.path:
        sys.path.append(_p)

import numpy as np
import ml_dtypes

# Problem constants (hardcoded per contract).
INPUT_LEN = 512
N_MEM = 128
OUT_LEN = 256
T_TOTAL = 65536
N_CORES = 8
T_LOC = T_TOTAL // N_CORES   # 8192 owned frames per core
HALO = 64                    # washout halo (see module docstring)
TW = 512                     # column tile width (one PSUM bank of fp32)
NPAIR = T_LOC // (2 * TW)    # 8 pairs of owned tiles per core
W_IN = HALO + T_LOC          # 8256
W1_SCALE = 8.0               # host multiplies W1 by this; ACT applies 1/8

F16_NP = np.float16
F8_NP = ml_dtypes.float8_e4m3fn


def _build_nc():
    import concourse.tile as tile
    from concourse import bacc, mybir
    from contextlib import ExitStack

    F32 = mybir.dt.float32
    F16 = mybir.dt.float16
    F8 = mybir.dt.float8e4
    SIG = mybir.ActivationFunctionType.Sigmoid
    MUL = mybir.AluOpType.mult
    ADD = mybir.AluOpType.add
    DR = mybir.MatmulPerfMode.DoubleRow

    nc = bacc.Bacc()
    # DRAM layouts are host-packed so every DMA is fully contiguous.
    # c8[:, 0:1024] = w1 packed; c8[:, 1024:1280] = halo x packed.
    c8 = nc.dram_tensor("c8", [128, 1024 + 4 * HALO], F8, kind="ExternalInput")
    # c16[:, 0:256] = w2 packed; c16[:, 256:264] = biases
    # [B1a, B1b, -B2a, -B2b, bscale, 0, 0, 0].
    c16 = nc.dram_tensor("c16", [128, 264], F16, kind="ExternalInput")
    x_main = nc.dram_tensor("x_main", [NPAIR, 128, 4096], F8, kind="ExternalInput")
    out = nc.dram_tensor("out", [NPAIR, 128, 2048], F16, kind="ExternalOutput")

    with ExitStack() as ctx:
        tc = ctx.enter_context(tile.TileContext(nc))
        singles = ctx.enter_context(tc.tile_pool(name="singles", bufs=1))
        xpool = ctx.enter_context(tc.tile_pool(name="xpool", bufs=4))
        hpool = ctx.enter_context(tc.tile_pool(name="hpool", bufs=4))
        opool = ctx.enter_context(tc.tile_pool(name="opool", bufs=6))
        # One 2-bank PSUM pool per distinct sigmoid bias: h-GEMM halves
        # (B1a/B1b) and out-GEMM halves (-B2a/-B2b).  Each tile holds both
        # 512-col t-tiles of a pair adjacently -> 1024-col ACT instructions.
        psG0 = ctx.enter_context(tc.tile_pool(name="psG0", bufs=1, space="PSUM"))
        psG1 = ctx.enter_context(tc.tile_pool(name="psG1", bufs=1, space="PSUM"))
        psO0 = ctx.enter_context(tc.tile_pool(name="psO0", bufs=1, space="PSUM"))
        psO1 = ctx.enter_context(tc.tile_pool(name="psO1", bufs=1, space="PSUM"))

        # Persistent recurrence buffers. a_buf/b_buf are written at a +1
        # column offset (a_buf[:, p] = a at input column p-1) so the scan
        # output z[:, p] = u at column p directly.
        a_buf = singles.tile([N_MEM, W_IN + 1], F16)
        b_buf = singles.tile([N_MEM, W_IN + 1], F16)
        z_buf = singles.tile([N_MEM, W_IN], F16)

        # ---- constants: two packed DMAs; pair-0 input is issue #2 so its
        # data lands as early as possible (each dma_start serializes ~600ns
        # on the Sync queue).
        c8_sb = singles.tile([128, 1024 + 4 * HALO], F8)
        nc.sync.dma_start(out=c8_sb, in_=c8[:])
        # w1_sb[p, c, i, h, m] = 8*W1[h*128+m, c*256 + i*128 + p]
        w1_sb = c8_sb[:, 0:1024].rearrange("p (c i h m) -> p c i h m", c=2, i=2, h=2)
        xh_sb = c8_sb[:, 1024:1024 + 4 * HALO].rearrange(
            "p (c i w) -> p c i w", c=2, i=2)

        xmr = x_main[:].rearrange("q p (t c i w) -> q p t c i w", t=2, c=2, i=2)
        outr = out[:].rearrange("q p (h w) -> q p h w", h=2)

        # Biases ride the small c16 DMA -- issue #2 so the halo activations
        # aren't bias-gated.  Pair 0 arrives as two half-loads so its first
        # 512-col GEMM (and sigmoid) starts before the full pair lands.
        c16_sb = singles.tile([128, 264], F16)
        nc.sync.dma_start(out=c16_sb, in_=c16[:])
        # w2_sb[p, h, m] = W2[h*128+m, p]
        w2_sb = c16_sb[:, 0:256].rearrange("p (h m) -> p h m", h=2)

        xt0 = xpool.tile([128, 2, 2, 2, TW], F8, name="xt")
        nc.sync.dma_start(out=xt0[:, 0], in_=xmr[0, :, 0])
        nc.sync.dma_start(out=xt0[:, 1], in_=xmr[0, :, 1])

        xt1 = xpool.tile([128, 2, 2, 2, TW], F8, name="xt")
        nc.sync.dma_start(out=xt1, in_=xmr[1])

        nc.vector.memset(a_buf[:, 0:1], 0.0)
        nc.vector.memset(b_buf[:, 0:1], 0.0)

        # DVFS warmup + ACT table preload: the PE starts in its lowest
        # p-state and ramps only while busy; the first Sigmoid pays a
        # 1283ns table load.  Both are hidden in the ~10us prologue dead
        # time (preamble + first DMAs).
        scratch = singles.tile([128, 2, TW], F8)
        nc.vector.memset(scratch, 0.0)
        warm_in = singles.tile([128, 1], F16)
        nc.gpsimd.memset(warm_in, 0.0)
        warm_act = singles.tile([128, 1], F16)
        nc.scalar.activation(warm_act, warm_in, SIG)
        for w in range(4):
            ow = (psO0 if w % 2 == 0 else psO1).tile([128, 2, TW], F32, name="o")
            for t in range(2 if w else 1):
                nc.tensor.matmul(ow[:, t, :], lhsT=scratch[:, :, 0:128],
                                 rhs=scratch, start=True, stop=True,
                                 perf_mode=DR)

        # Widen f16 biases to f32 once (ACT bias/scale operands read f32).
        bias_sb = singles.tile([128, 8], F32)
        nc.vector.tensor_scalar(out=bias_sb, in0=c16_sb[:, 256:264],
                                scalar1=1.0, scalar2=None, op0=MUL)

        def phase_c(q, pools=None, names=("o", "o")):
            # output GEMM + activation + store for pair q (z cols
            # [HALO+1024q, HALO+1024q+1024), out cols [1024q, 1024q+1024))
            zc = HALO + 1024 * q
            if pools is None:
                pools = (psO0, psO1)
            for h, pool in ((0, pools[0]), (1, pools[1])):
                o = pool.tile([128, 2, TW], F32, name=names[h])
                for t in range(2):
                    nc.tensor.matmul(o[:, t, :], lhsT=w2_sb[:, h, :],
                                     rhs=z_buf[:, zc + TW * t:zc + TW * (t + 1)],
                                     start=True, stop=True)
                ot = opool.tile([128, 1024], F16, name="ot")
                nc.scalar.activation(ot, o[:].rearrange("p t w -> p (t w)"),
                                     SIG, bias=bias_sb[:, 2 + h:3 + h],
                                     scale=-1.0)
                nc.sync.dma_start(out=outr[q, :, h, :], in_=ot)

        # ---- halo tile (columns [0, HALO)) ----
        gh = [psG0.tile([128, 2, TW], F32, name="g"),
              psG1.tile([128, 2, TW], F32, name="g")]
        for h in range(2):
            for c in range(2):
                nc.tensor.matmul(gh[h][:, 0, 0:HALO], lhsT=w1_sb[:, c, :, h, :],
                                 rhs=xh_sb[:, c, :, :],
                                 start=(c == 0), stop=(c == 1), perf_mode=DR)
        lrh = hpool.tile([128, 2, HALO], F16)
        for h in range(2):
            nc.scalar.activation(lrh[:, h, :], gh[h][:, 0, 0:HALO], SIG,
                                 bias=bias_sb[:, h:h + 1], scale=1.0 / W1_SCALE)
        nc.gpsimd.tensor_scalar(out=b_buf[:, 1:1 + HALO],
                                in0=lrh[:, 0, :],
                                scalar1=-1.0, scalar2=1.0, op0=MUL, op1=ADD)
        nc.vector.tensor_tensor(out=a_buf[:, 1:1 + HALO],
                                in0=lrh[:, 0, :],
                                in1=lrh[:, 1, :], op=MUL)
        # Halo b *= bscale (0 on core 0 so the state stays exactly 0)
        nc.vector.tensor_scalar(out=b_buf[:, 0:HALO + 1],
                                in0=b_buf[:, 0:HALO + 1],
                                scalar1=bias_sb[:, 4:5], scalar2=None, op0=MUL)
        nc.vector.tensor_tensor_scan(out=z_buf[:, 0:HALO],
                                     data0=a_buf[:, 0:HALO],
                                     data1=b_buf[:, 0:HALO],
                                     initial=0.0, op0=MUL, op1=ADD)

        DELAY = 2  # pairs of lead distance between phase A/B and phase C

        # ---- owned pairs ----
        for p in range(NPAIR):
            c0 = HALO + 1024 * p
            if p == 0:
                xt = xt0
            elif p == 1:
                xt = xt1
            else:
                xt = xpool.tile([128, 2, 2, 2, TW], F8, name="xt")
                nc.sync.dma_start(out=xt, in_=xmr[p])
            lr = hpool.tile([128, 2, 1024], F16)
            if p <= 1:
                # Pipeline fill: each (h, t) quarter gets its OWN PSUM tile
                # (borrowing the psO rings, idle until phase_c(0) at p=2)
                # so each 512-col act fires right after its two matmuls --
                # tile deps are whole-tile, so sharing a tile would delay
                # the first sigmoid by ~1.7us.  Pairs 0-1 run while the PE
                # is still at mid-DVFS, so earlier acts fill ramp gaps.
                for h, t, pool, nm in ((0, 0, psG0, "g"), (0, 1, psO0, "o"),
                                       (1, 0, psG1, "g"), (1, 1, psO1, "o")):
                    g = pool.tile([128, 2, TW], F32, name=nm)
                    for c in range(2):
                        nc.tensor.matmul(
                            g[:, 0, :], lhsT=w1_sb[:, c, :, h, :],
                            rhs=xt[:, t, c, :, :],
                            start=(c == 0), stop=(c == 1), perf_mode=DR)
                    nc.scalar.activation(lr[:, h, TW * t:TW * (t + 1)],
                                         g[:, 0, :], SIG,
                                         bias=bias_sb[:, h:h + 1],
                                         scale=1.0 / W1_SCALE)
            else:
                # h-major GEMM order: ACT-h0 fires after the first two
                # groups while h1 still computes; per-h PSUM pools stagger
                # the WARs.
                for h, pool in ((0, psG0), (1, psG1)):
                    g = pool.tile([128, 2, TW], F32, name="g")
                    for t in range(2):
                        for c in range(2):
                            nc.tensor.matmul(
                                g[:, t, :], lhsT=w1_sb[:, c, :, h, :],
                                rhs=xt[:, t, c, :, :],
                                start=(c == 0), stop=(c == 1), perf_mode=DR)
                    nc.scalar.activation(lr[:, h, :],
                                         g[:].rearrange("p t w -> p (t w)"),
                                         SIG, bias=bias_sb[:, h:h + 1],
                                         scale=1.0 / W1_SCALE)
            nc.gpsimd.tensor_scalar(out=b_buf[:, c0 + 1:c0 + 1025],
                                    in0=lr[:, 0, :],
                                    scalar1=-1.0, scalar2=1.0,
                                    op0=MUL, op1=ADD)
            if p < NPAIR - 1:
                nc.vector.tensor_tensor(out=a_buf[:, c0 + 1:c0 + 1025],
                                        in0=lr[:, 0, :], in1=lr[:, 1, :],
                                        op=MUL)
                nc.vector.tensor_tensor_scan(out=z_buf[:, c0:c0 + 1024],
                                             data0=a_buf[:, c0:c0 + 1024],
                                             data1=b_buf[:, c0:c0 + 1024],
                                             initial=z_buf[:, c0 - 1:c0],
                                             op0=MUL, op1=ADD)
                if p - DELAY >= 0:
                    phase_c(p - DELAY)
            else:
                # Final pair: split a/scan into halves so the tail output
                # chain starts half a pair earlier.
                nc.vector.tensor_tensor(out=a_buf[:, c0 + 1:c0 + TW + 1],
                                        in0=lr[:, 0, 0:TW], in1=lr[:, 1, 0:TW],
                                        op=MUL)
                nc.vector.tensor_tensor_scan(out=z_buf[:, c0:c0 + TW],
                                             data0=a_buf[:, c0:c0 + TW],
                                             data1=b_buf[:, c0:c0 + TW],
                                             initial=z_buf[:, c0 - 1:c0],
                                             op0=MUL, op1=ADD)
                nc.vector.tensor_tensor(out=a_buf[:, c0 + TW + 1:c0 + 1025],
                                        in0=lr[:, 0, TW:1024],
                                        in1=lr[:, 1, TW:1024], op=MUL)
                phase_c(p - DELAY)
                nc.vector.tensor_tensor_scan(
                    out=z_buf[:, c0 + TW:c0 + 1024],
                    data0=a_buf[:, c0 + TW:c0 + 1024],
                    data1=b_buf[:, c0 + TW:c0 + 1024],
                    initial=z_buf[:, c0 + TW - 1:c0 + TW],
                    op0=MUL, op1=ADD)
                # Pair 6 out-GEMMs run on the psG rings (free right after
                # pair 7's h-acts) instead of waiting for pair-5's
                # out-acts to release the psO banks.
                phase_c(p - 1, pools=(psG0, psG1), names=("g", "g"))
                # Pair 7 out-GEMMs on the psO rings (free after out(5));
                # 512-col acts + 128KB stores so each quarter drains as
                # soon as its scan half + GEMM are done.
                of = [psO0.tile([128, 2, TW], F32, name="o"),
                      psO1.tile([128, 2, TW], F32, name="o")]
                otf = [opool.tile([128, 1024], F16, name="ot"),
                       opool.tile([128, 1024], F16, name="ot")]
                outq = out[:].rearrange("q p (h t w) -> q p h t w", h=2, t=2)
                for t in range(2):
                    for h in range(2):
                        nc.tensor.matmul(
                            of[h][:, t, :], lhsT=w2_sb[:, h, :],
                            rhs=z_buf[:, c0 + TW * t:c0 + TW * (t + 1)],
                            start=True, stop=True)
                for t in range(2):
                    for h in range(2):
                        nc.scalar.activation(otf[h][:, TW * t:TW * (t + 1)],
                                             of[h][:, t, :], SIG,
                                             bias=bias_sb[:, 2 + h:3 + h],
                                             scale=-1.0)
                        nc.sync.dma_start(out=outq[p, :, h, t, :],
                                          in_=otf[h][:, TW * t:TW * (t + 1)])

    nc.finalize()
    return nc


def _make_in_maps(inputs, W1, B1, W2, B2):
    inputs = np.asarray(inputs, dtype=np.float32)
    W1 = np.asarray(W1, dtype=np.float32)
    B1 = np.asarray(B1, dtype=np.float32)
    W2 = np.asarray(W2, dtype=np.float32)
    B2 = np.asarray(B2, dtype=np.float32)

    x8 = inputs.astype(F8_NP)
    # w1[p, c, i, h, m] = 8*W1[h*128+m, c*256+i*128+p]
    w1p = np.ascontiguousarray(
        (W1 * W1_SCALE).astype(F8_NP)
        .reshape(2, 128, 2, 2, 128)            # h, m, c, i, p
        .transpose(4, 2, 3, 0, 1)              # p, c, i, h, m
        .reshape(128, 1024))
    # w2[p, h, m] = W2[h*128+m, p]
    w2p = np.ascontiguousarray(
        W2.astype(F16_NP).reshape(2, 128, 128)  # h, m, p
        .transpose(2, 0, 1).reshape(128, 256))
    biasc = np.zeros((128, 8), np.float16)
    biasc[:, 0] = B1[:128, 0].astype(np.float16)
    biasc[:, 1] = B1[128:, 0].astype(np.float16)
    biasc[:, 2] = (-B2[:128, 0]).astype(np.float16)
    biasc[:, 3] = (-B2[128:, 0]).astype(np.float16)

    in_maps = []
    for i in range(N_CORES):
        s = i * T_LOC
        lo = s - HALO
        if lo < 0:
            xs = np.concatenate(
                [np.zeros((INPUT_LEN, -lo), F8_NP), x8[:, :s + T_LOC]], axis=1)
        else:
            xs = x8[:, lo:s + T_LOC]
        xr = xs.reshape(2, 2, 128, W_IN)                  # c, i, p, col
        xhm = np.ascontiguousarray(
            xr[:, :, :, :HALO].transpose(2, 0, 1, 3).reshape(128, 4 * HALO))
        xm = np.ascontiguousarray(
            xr[:, :, :, HALO:].reshape(2, 2, 128, NPAIR, 2, TW)
            .transpose(3, 2, 4, 0, 1, 5)                  # pair, p, t, c, i, w
            .reshape(NPAIR, 128, 4096))
        b = biasc.copy()
        b[:, 4] = 0.0 if i == 0 else 1.0
        c8p = np.ascontiguousarray(np.concatenate([w1p, xhm], axis=1))
        c16p = np.ascontiguousarray(
            np.concatenate([w2p, b.astype(F16_NP)], axis=1))
        in_maps.append({
            "c8": c8p, "c16": c16p, "x_main": xm,
        })
    return in_maps


def _run(inputs, W1, B1, W2, B2, trace=False, **kw):
    from concourse.bass_utils import run_bass_kernel_spmd

    nc = _build_nc()
    in_maps = _make_in_maps(inputs, W1, B1, W2, B2)
    res = run_bass_kernel_spmd(nc, in_maps, list(range(N_CORES)), trace=trace, **kw)
    parts = []
    for r in res.results:
        o = np.asarray(r["out"]).astype(np.float32)       # (NPAIR, 128, 2048)
        o = (o.reshape(NPAIR, 128, 2, 2, TW)              # pair, p, h, t, w
             .transpose(2, 1, 0, 3, 4)                    # h, p, pair, t, w
             .reshape(OUT_LEN, T_LOC))
        parts.append(o)
    full = np.concatenate(parts, axis=1)
    return full, res


def kernel(inputs, W1, B1, W2, B2):
    full, _ = _run(inputs, W1, B1, W2, B2, trace=False)
    return full.astype(np.float32, copy=False)
